# revision 7
# baseline (speedup 1.0000x reference)
"""Dynamic structural masking attention on 8 Trainium2 NeuronCores.

Reference computation (per batch b):
    sim  = cos_sim(x, x)                      [S, S]
    mask = sim > 0.7                          (shared across heads)
    q/k/v = x @ W.T + b, per-head split
    out  = softmax(where(mask, q k^T / 8, -inf)) @ v   [H, S, dk]

Sharding over 8 cores: batch (2) x head-group (2) x query-slice (2); each
core computes 8 heads x 1024 queries over all 2048 keys.

All matmuls run in fp8-e4m3 DoubleRow perf mode (2x128-deep contraction at
0.5 cycles/row = 4x the fp32r MAC rate):
  - Gram/mask: x in fp8; norms from fp8 squares (ones-matmul reduce); the
    [q,q] block's below-diagonal spans filled by bf16 xbar DMA transposes.
  - Q/K projections in single fp8 (score errors cancel row-globally in
    softmax); V in split precision (x_hi*w_hi + x_hi*w_lo + x_lo*w_hi,
    ~bf16 accuracy), + bias via a rank-1 [1,2,...] DoubleRow matmul.
  - Scores K^T Q with dk=64 packed as [32,2] (DMA partition-remap of the
    projection outputs); attention weights p = e^{s/8 - C} (C=5 cancels in
    the softmax normalization) stored fp8.
  - AV pairs (v_hi, v_lo) as the DoubleRow halves with p broadcast across
    halves by a 0-stride AP; the softmax denominator rides along as a ones
    column; a delta*I pass adds delta*(v_hi+v_lo) / delta to num/den so
    singleton-mask rows are exact even when p underflows fp8.
The exp+mask is balanced across three engines: most chunks are a single
DVE scalar_tensor_tensor computing round(1.4427*s + M) saturated to uint8
and bitcast as fp8 (the exp2 exponent-field trick; M = -1.7078 unmasked /
-448 masked built into the mask tiles), the rest get ACT Exp->fp8 plus a
mask multiply on Pool (gpsimd) or DVE. Final normalization (num/den) is
done on the host. Verified on hardware vs the fp32 reference.
"""

import numpy as np

# Problem dims (hardcoded per contract; kernel.py must be self-contained).
B = 2
S = 2048
D = 1024
H_TOT = 16
DK = 64
SIM_THRESH = 0.7
N_CORES = 8

_CACHE = {}

# fp8 exp encoding constants (C = 5.0 global downshift, cancels per-row)
C_SHIFT = 5.0
LOG2E = 1.4426950408889634
B_SOLO = 56.0 - 8.0 * C_SHIFT * LOG2E      # -1.70780 (unmasked add-form)
M_MASKED = -448.0
AFF_SCALE = B_SOLO - M_MASKED              # 446.2922
EXP_BIAS_PSUM = -C_SHIFT                   # ACT path from raw scores
DELTA = 0.015625                           # 2^-6: smallest NORMAL fp8 (PE flushes subnormals)

# stage-D flavor tables (tunable): tcn in ADD_TCNS -> add-form mask,
# chunks there use the single-op DVE stt->uint8 path; other tcns keep the
# 0/1 mask and use ACT exp + multiply on Pool (or DVE for DVEMULT pairs).
ADD_TCNS = frozenset(range(0, 9))
DVEMULT = frozenset((h, 15) for h in (1, 3, 5, 7))


def _build(n_cores=N_CORES):
    import concourse.bacc as bacc
    import concourse.mybir as mybir
    import concourse.tile as tile

    f32 = mybir.dt.float32
    bf16 = mybir.dt.bfloat16
    fp8 = mybir.dt.float8e4
    u8 = mybir.dt.uint8
    Alu = mybir.AluOpType
    Act = mybir.ActivationFunctionType
    DR = mybir.MatmulPerfMode.DoubleRow

    H_LOC = 8
    SQ = 1024
    JH = H_LOC * DK          # 512
    NT = S // 128            # 16 key chunks
    NP = D // 256            # 4 d-pairs
    NJ = JH // 128           # 4 projection col chunks
    NSP = SQ // 512          # 2 query spans
    NKS = S // 512           # 4 key spans

    nc = bacc.Bacc("TRN2", target_bir_lowering=False, debug=False,
                   num_devices=n_cores)

    x8h_d = nc.dram_tensor("x8h", [NP, 128, 2, S], fp8, kind="ExternalInput")
    x8l_d = nc.dram_tensor("x8l", [NP, 128, 2, S], fp8, kind="ExternalInput")
    w8q_d = nc.dram_tensor("w8q", [NP, 128, 2, JH], fp8, kind="ExternalInput")
    w8k_d = nc.dram_tensor("w8k", [NP, 128, 2, JH], fp8, kind="ExternalInput")
    w8vh_d = nc.dram_tensor("w8vh", [NP, 128, 2, JH], fp8, kind="ExternalInput")
    w8vl_d = nc.dram_tensor("w8vl", [NP, 128, 2, JH], fp8, kind="ExternalInput")
    bq_d = nc.dram_tensor("bq128", [128, NJ], f32, kind="ExternalInput")
    bk_d = nc.dram_tensor("bk128", [128, NJ], f32, kind="ExternalInput")
    bv8_d = nc.dram_tensor("bv8", [1, 2, JH], fp8, kind="ExternalInput")
    dd_d = nc.dram_tensor("dd", [128, 2, 128], fp8, kind="ExternalInput")
    out_d = nc.dram_tensor("out", [H_LOC, 65, SQ], f32, kind="ExternalOutput")

    with tile.TileContext(nc) as tc:
        with (
            tc.tile_pool(name="xin", bufs=4) as xin,
            tc.tile_pool(name="wts", bufs=4) as wts,
            tc.tile_pool(name="mask", bufs=16) as mpool,
            tc.tile_pool(name="qk8", bufs=4) as qk8,
            tc.tile_pool(name="vhl", bufs=16) as vpool,
            tc.tile_pool(name="lin", bufs=2) as linp,
            tc.tile_pool(name="p8", bufs=8) as p8pool,
            tc.tile_pool(name="e8", bufs=3) as e8pool,
            tc.tile_pool(name="ot", bufs=2) as otpool,
            tc.tile_pool(name="small", bufs=1) as small,
            tc.tile_pool(name="sqp", bufs=4) as sqp,
            tc.tile_pool(name="dram", bufs=1, space="DRAM") as dram,
            tc.tile_pool(name="scps", bufs=2, space="PSUM") as scps,
            tc.tile_pool(name="avps", bufs=1, space="PSUM") as avps,
            tc.tile_pool(name="aux", bufs=2, space="PSUM") as aux,
        ):
            # ---- persistent tiles ----
            x8h = [xin.tile([128, 2, S], fp8, tag="x8h", name=f"x8h{i}")
                   for i in range(NP)]
            x8l = [xin.tile([128, 2, S], fp8, tag="x8l", name=f"x8l{i}")
                   for i in range(NP)]
            w8q = [wts.tile([128, 2, JH], fp8, tag="w8q", name=f"w8q{i}")
                   for i in range(NP)]
            w8k = [wts.tile([128, 2, JH], fp8, tag="w8k", name=f"w8k{i}")
                   for i in range(NP)]
            w8vh = [wts.tile([128, 2, JH], fp8, tag="w8vh", name=f"w8vh{i}")
                    for i in range(NP)]
            w8vl = [wts.tile([128, 2, JH], fp8, tag="w8vl", name=f"w8vl{i}")
                    for i in range(NP)]
            mask_t = [mpool.tile([128, SQ], bf16, tag="mask", name=f"mask{t}")
                      for t in range(NT)]
            qt8 = [qk8.tile([64, 2, SQ], fp8, tag="qt8", name=f"qt8_{t}")
                   for t in range(4)]
            kt8 = [qk8.tile([64, 2, S], fp8, tag="kt8", name=f"kt8_{t}")
                   for t in range(4)]
            vhl = [vpool.tile([128, 2, H_LOC, 66], fp8, tag="vhl",
                              name=f"vhl{t}") for t in range(NT)]
            bq_t = small.tile([128, NJ], f32, tag="bq")
            bk_t = small.tile([128, NJ], f32, tag="bk")
            bv8_t = small.tile([1, 2, JH], fp8, tag="bv8")
            dd_t = small.tile([128, 2, 128], fp8, tag="dd")
            ones16 = small.tile([128, 2, 16], fp8, tag="ones16")
            onesbv = small.tile([1, 2, 128], fp8, tag="onesbv")
            nk_row = small.tile([1, S], f32, tag="nkrow")
            thrq_row = small.tile([1, SQ], f32, tag="thrqrow")
            invnk_cols = small.tile([128, NT], f32, tag="invnk")
            thrq_bc = small.tile([128, SQ], f32, tag="thrqbc")
            neg448_t = small.tile([128, 1], f32, tag="neg448")
            bias5_t = small.tile([128, 1], f32, tag="bias5")
            sq8 = [sqp.tile([128, 2, S], fp8, tag="sq8", name=f"sq8_{i}")
                   for i in range(NP)]
            dscr = dram.tile([1, S], f32, tag="dscr")

            # ---- input DMAs (x8h first: everything needs it) ----
            for i in range(NP):
                nc.sync.dma_start(x8h[i][:], x8h_d.ap()[i])
            for i in range(NP):
                nc.gpsimd.dma_start(w8q[i][:], w8q_d.ap()[i])
                nc.gpsimd.dma_start(w8k[i][:], w8k_d.ap()[i])
            nc.gpsimd.dma_start(bq_t[:], bq_d.ap())
            nc.gpsimd.dma_start(bk_t[:], bk_d.ap())
            nc.gpsimd.dma_start(bv8_t[:], bv8_d.ap())
            nc.gpsimd.dma_start(dd_t[:], dd_d.ap())
            for i in range(NP):
                nc.gpsimd.dma_start(w8vh[i][:], w8vh_d.ap()[i])
                nc.sync.dma_start(x8l[i][:], x8l_d.ap()[i])
                nc.gpsimd.dma_start(w8vl[i][:], w8vl_d.ap()[i])

            nc.vector.memset(ones16[:], 1.0)
            nc.vector.memset(onesbv[:], 1.0)
            nc.vector.memset(neg448_t[:], M_MASKED)
            nc.vector.memset(bias5_t[:], EXP_BIAS_PSUM)
            for t in range(NT):
                nc.gpsimd.memset(vhl[t][:, 0, :, 64:65], 1.0)
                nc.gpsimd.memset(vhl[t][:, 0, :, 65:66], 0.0)
                nc.gpsimd.memset(vhl[t][:, 1, :, 64:66], 0.0)

            # ---- norms: |x|, 0.7|x|, 1/|x| from fp8 squares ----
            def emit_norms():
                for i in range(NP):
                    # x8h holds 16*x: Square(in/16) = x^2
                    nc.scalar.activation(sq8[i][:], x8h[i][:], Act.Square,
                                         scale=0.0625)
                for ks in range(NKS):
                    n_ps = aux.tile([128, 512], f32, tag="aux",
                                    name=f"nps{ks}")
                    for i in range(NP):
                        nc.tensor.matmul(
                            n_ps[0:16, :], ones16[:],
                            x8l[0][:, :, 0:512] if False else
                            sq8[i][:, :, ks * 512:(ks + 1) * 512],
                            start=(i == 0), stop=(i == NP - 1), perf_mode=DR)
                    # gram psum carries (16x).(16x) = 256*G, so invnk must
                    # be 1/(256|x|): sqrt(65536*n2) = 256|x|
                    nc.scalar.activation(
                        nk_row[0:1, ks * 512:(ks + 1) * 512],
                        n_ps[0:1, :], Act.Sqrt, scale=65536.0)
                    if ks < NSP:
                        nc.scalar.activation(
                            thrq_row[0:1, ks * 512:(ks + 1) * 512],
                            n_ps[0:1, :], Act.Sqrt,
                            scale=SIM_THRESH * SIM_THRESH)
                nc.vector.reciprocal(nk_row[0:1, :], nk_row[0:1, :])
                nc.sync.dma_start(dscr[0:1, :], nk_row[0:1, :])
                nc.sync.dma_start(
                    invnk_cols[:],
                    dscr[0:1, :].rearrange("o (c p) -> (o p) c", p=128))
                nc.gpsimd.partition_broadcast(thrq_bc[:], thrq_row[:])

            # ---- Gram chunk -> mask tile (0/1 or add-form) ----
            def emit_gram(t):
                sav = t // 4 if t < 8 else 0
                col0 = sav * 512
                for a in range(col0, SQ, 512):
                    g_ps = aux.tile([128, 512], f32, tag="aux",
                                    name=f"gps{t}_{a}")
                    for i in range(NP):
                        nc.tensor.matmul(
                            g_ps[:], x8h[i][:, :, t * 128:(t + 1) * 128],
                            x8h[i][:, :, a:a + 512],
                            start=(i == 0), stop=(i == NP - 1), perf_mode=DR)
                    nc.vector.scalar_tensor_tensor(
                        mask_t[t][:, a:a + 512], g_ps[:],
                        invnk_cols[:, t:t + 1], thrq_bc[:, a:a + 512],
                        op0=Alu.mult, op1=Alu.is_gt)
                    if t in ADD_TCNS:
                        nc.scalar.activation(
                            mask_t[t][:, a:a + 512], mask_t[t][:, a:a + 512],
                            Act.Identity, scale=AFF_SCALE, bias=neg448_t[:])
                for m in range(4 * sav):
                    nc.sync.dma_start(
                        mask_t[t][:, m * 128:(m + 1) * 128],
                        mask_t[m][:, t * 128:(t + 1) * 128],
                        transpose=True)

            # ---- Q projection (+ remap to [32,2] layout) ----
            def emit_q(jc):
                qlin = linp.tile([128, SQ], fp8, tag="qlin",
                                 name=f"qlin{jc}")
                for sp in range(NSP):
                    q_ps = aux.tile([128, 512], f32, tag="aux",
                                    name=f"qps{jc}_{sp}")
                    for i in range(NP):
                        nc.tensor.matmul(
                            q_ps[:], w8q[i][:, :, jc * 128:(jc + 1) * 128],
                            x8h[i][:, :, sp * 512:(sp + 1) * 512],
                            start=(i == 0), stop=(i == NP - 1), perf_mode=DR)
                    nc.scalar.activation(
                        qlin[:, sp * 512:(sp + 1) * 512], q_ps[:],
                        Act.Identity, scale=0.0625, bias=bq_t[:, jc:jc + 1])
                for e in range(2):
                    h = 2 * jc + e
                    for half in range(2):
                        nc.gpsimd.dma_start(
                            qt8[h // 2][32 * (h % 2):32 * (h % 2) + 32,
                                        half, :],
                            qlin[e * 64 + half * 32:e * 64 + half * 32 + 32,
                                 :])

            # ---- K projection (+ remap) ----
            def emit_k(jc):
                klin = linp.tile([128, S], fp8, tag="klin", name=f"klin{jc}")
                for ks in range(NKS):
                    k_ps = aux.tile([128, 512], f32, tag="aux",
                                    name=f"kps{jc}_{ks}")
                    for i in range(NP):
                        nc.tensor.matmul(
                            k_ps[:], w8k[i][:, :, jc * 128:(jc + 1) * 128],
                            x8h[i][:, :, ks * 512:(ks + 1) * 512],
                            start=(i == 0), stop=(i == NP - 1), perf_mode=DR)
                    nc.scalar.activation(
                        klin[:, ks * 512:(ks + 1) * 512], k_ps[:],
                        Act.Identity, scale=0.0625, bias=bk_t[:, jc:jc + 1])
                for e in range(2):
                    h = 2 * jc + e
                    for half in range(2):
                        nc.gpsimd.dma_start(
                            kt8[h // 2][32 * (h % 2):32 * (h % 2) + 32,
                                        half, :],
                            klin[e * 64 + half * 32:e * 64 + half * 32 + 32,
                                 :])

            # ---- V chunk: hi/lo split with ones column ----
            def emit_v(sc):
                v_ps = aux.tile([128, 512], f32, tag="aux", name=f"vps{sc}")
                first = True
                for xa, wb in ((x8h, w8vh), (x8h, w8vl), (x8l, w8vh)):
                    for i in range(NP):
                        nc.tensor.matmul(
                            v_ps[:], xa[i][:, :, sc * 128:(sc + 1) * 128],
                            wb[i][:], start=first, stop=False, perf_mode=DR)
                        first = False
                nc.tensor.matmul(v_ps[:], onesbv[:], bv8_t[:],
                                 start=False, stop=True, perf_mode=DR)
                # v_ps carries (16x).(64w) = 1024*v; rescale on eviction
                vr = v_ps[:].rearrange("p (h e) -> p h e", h=H_LOC)
                nc.scalar.activation(vhl[sc][:, 0, :, 0:64], vr, Act.Identity,
                                     scale=0.0009765625)
                nc.vector.scalar_tensor_tensor(
                    vhl[sc][:, 1, :, 0:64], vr, 0.0009765625,
                    vhl[sc][:, 0, :, 0:64], op0=Alu.mult, op1=Alu.subtract)

            # ---- stage D ----
            def emit_scores(h, t):
                s_ps = scps.tile([128, SQ], f32, tag="sc", name=f"sps{h}_{t}")
                hh = h % 2
                for sp in range(NSP):
                    nc.tensor.matmul(
                        s_ps[:, sp * 512:(sp + 1) * 512],
                        kt8[h // 2][32 * hh:32 * hh + 32, :,
                                    t * 128:(t + 1) * 128],
                        qt8[h // 2][32 * hh:32 * hh + 32, :,
                                    sp * 512:(sp + 1) * 512],
                        start=True, stop=True, perf_mode=DR)
                p8t = p8pool.tile([128, SQ], fp8, tag="p8", name=f"p8_{h}_{t}")
                if t in ADD_TCNS:
                    nc.vector.scalar_tensor_tensor(
                        p8t[:].bitcast(u8), s_ps[:], LOG2E, mask_t[t][:],
                        op0=Alu.mult, op1=Alu.add)
                else:
                    e8t = e8pool.tile([128, SQ], fp8, tag="e8",
                                      name=f"e8_{h}_{t}")
                    nc.scalar.activation(e8t[:], s_ps[:], Act.Exp,
                                         scale=0.125, bias=bias5_t[:])
                    eng = nc.vector if (h, t) in DVEMULT else nc.gpsimd
                    eng.tensor_tensor(p8t[:], e8t[:], mask_t[t][:],
                                      op=Alu.mult)
                return p8t

            def emit_av(h, t, p8t, av_t, first):
                for sp in range(NSP):
                    rhs = p8t[:, sp * 512:(sp + 1) * 512].rearrange(
                        "q (two f) -> q two f", two=1).broadcast_to(
                        (128, 2, 512))
                    nc.tensor.matmul(
                        av_t[:, sp * 512:(sp + 1) * 512],
                        vhl[t][:, :, h, :], rhs,
                        start=first, stop=False,
                        perf_mode=DR, skip_group_check=True)

            def emit_delta(h, av_t):
                # runs AFTER all AV chunks: start=True would zero the whole
                # 512-wide psum zero-region, wiping sibling delta blocks.
                for sc in range(8):
                    nc.tensor.matmul(
                        av_t[0:66, sc * 128:(sc + 1) * 128],
                        vhl[sc][:, :, h, :], dd_t[:],
                        start=False, stop=(sc % 4 == 3),
                        perf_mode=DR, skip_group_check=True)

            def emit_epilogue(h, av_t):
                o_t = otpool.tile([65, SQ], f32, tag="ot", name=f"ot{h}")
                nc.scalar.activation(o_t[:], av_t[0:65, :], Act.Identity)
                nc.sync.dma_start(out_d.ap()[h], o_t[:])

            # ---- emission schedule ----
            emit_norms()
            emit_q(0)
            emit_q(1)
            for t in range(6):
                emit_gram(t)
            emit_k(0)
            for sc in range(4):
                emit_v(sc)
            emit_q(2)
            emit_q(3)

            prework = {h: [] for h in range(H_LOC)}
            pre0 = prework[0]
            for t in range(6, NT):
                pre0.append(lambda t=t: emit_gram(t))
                sc = t - 2
                if sc < NT:
                    pre0.append(lambda sc=sc: emit_v(sc))
            pre0.append(lambda: emit_v(14))
            pre0.append(lambda: emit_v(15))
            pre0.append(lambda: emit_k(1))
            prework[1].append(lambda: emit_k(2))
            prework[3].append(lambda: emit_k(3))

            LAG = 6
            for h in range(H_LOC):
                work = prework[h]
                av_t = avps.tile([66, SQ], f32, tag="av", name=f"av{h}")
                pending = []
                for t in range(NT):
                    for _ in range(2):
                        if work:
                            work.pop(0)()
                    p8t = emit_scores(h, t)
                    pending.append((t, p8t))
                    if len(pending) > LAG:
                        tt, pp = pending.pop(0)
                        emit_av(h, tt, pp, av_t, first=(tt == 0))
                while pending:
                    tt, pp = pending.pop(0)
                    emit_av(h, tt, pp, av_t, first=(tt == 0))
                emit_delta(h, av_t)
                emit_epilogue(h, av_t)

    nc.compile()
    return nc


def _get_nc():
    key = (S, D, H_TOT, SIM_THRESH)
    if key not in _CACHE:
        _CACHE[key] = _build()
    return _CACHE[key]


def _to_pairs(a):
    """[D, N] -> [D/256, 128, 2, N] pair layout."""
    d, n = a.shape
    return np.ascontiguousarray(
        a.reshape(d // 256, 2, 128, n).transpose(0, 2, 1, 3))


def make_in_maps(x, Wq, bq, Wk, bk, Wv, bv, h_loc=8, sq=1024, n_cores=N_CORES):
    """Per-core input dicts. Core c: batch, head-group, query-slice; its
    keys are rolled so the query slice comes first. Host work is dtype
    conversion + layout only."""
    import ml_dtypes
    F8 = ml_dtypes.float8_e4m3

    x = np.asarray(x, dtype=np.float32)
    Wq, Wk, Wv = (np.asarray(w, dtype=np.float32) for w in (Wq, Wk, Wv))
    bq, bk, bv = (np.asarray(v_, dtype=np.float32) for v_ in (bq, bk, bv))
    jh = h_loc * DK
    seq, d_model = x.shape[1], x.shape[2]
    n_hg = d_model // jh
    n_qs = seq // sq

    dd = np.zeros((128, 2, 128), np.float32)
    for r in range(128):
        dd[r, 0, r] = DELTA
        dd[r, 1, r] = DELTA
    dd = dd.astype(F8)

    in_maps = []
    for c in range(n_cores):
        b = c // (n_hg * n_qs)
        hg = (c % (n_hg * n_qs)) // n_qs
        qs = c % n_qs
        xb = x[b]
        order = np.concatenate([
            np.arange(qs * sq, (qs + 1) * sq),
            np.delete(np.arange(seq), np.s_[qs * sq:(qs + 1) * sq])])
        # scale splits into fp8's normal range (subnormals quantize badly
        # and the PE flushes them): x*16, Wv*64, bv*1024; undone by ACT
        # eviction scales on device.
        xt = np.ascontiguousarray(xb[order].T) * 16.0   # [D, S]
        xh8 = xt.astype(F8)
        xl8 = (xt - xh8.astype(np.float32)).astype(F8)

        wqt = np.ascontiguousarray(Wq[hg * jh:(hg + 1) * jh].T)
        wkt = np.ascontiguousarray(Wk[hg * jh:(hg + 1) * jh].T)
        wvt = np.ascontiguousarray(Wv[hg * jh:(hg + 1) * jh].T) * 64.0
        wvh8 = wvt.astype(F8)
        wvl8 = (wvt - wvh8.astype(np.float32)).astype(F8)

        bqs = bq[hg * jh:(hg + 1) * jh]
        bks = bk[hg * jh:(hg + 1) * jh]
        bvs = bv[hg * jh:(hg + 1) * jh] * 1024.0
        bvh8 = bvs.astype(F8)
        bvl8 = (bvs - bvh8.astype(np.float32)).astype(F8)

        in_maps.append({
            "x8h": _to_pairs(xh8),
            "x8l": _to_pairs(xl8),
            "w8q": _to_pairs(wqt.astype(F8)),
            "w8k": _to_pairs(wkt.astype(F8)),
            "w8vh": _to_pairs(wvh8),
            "w8vl": _to_pairs(wvl8),
            "bq128": np.ascontiguousarray(bqs.reshape(4, 128).T),
            "bk128": np.ascontiguousarray(bks.reshape(4, 128).T),
            "bv8": np.ascontiguousarray(
                np.stack([bvh8, bvl8], axis=0)[None]),
            "dd": dd,
        })
    return in_maps


def assemble(results, h_tot=H_TOT, seq=S, h_loc=8, sq=1024, n_cores=N_CORES):
    n_hg = h_tot // h_loc
    n_qs = seq // sq
    n_b = n_cores // (n_hg * n_qs)
    out = np.empty((n_b, h_tot, seq, DK), np.float32)
    for c in range(n_cores):
        b = c // (n_hg * n_qs)
        hg = (c % (n_hg * n_qs)) // n_qs
        qs = c % n_qs
        r = results[c]["out"]                       # [8, 65, SQ]
        att = r[:, 0:64, :] / r[:, 64:65, :]        # host normalize
        out[b, hg * h_loc:(hg + 1) * h_loc, qs * sq:(qs + 1) * sq, :] = \
            att.transpose(0, 2, 1)
    return out


def kernel(x, Wq, bq, Wk, bk, Wv, bv, _trace=False):
    from concourse.bass_utils import run_bass_kernel_spmd
    nc = _get_nc()
    in_maps = make_in_maps(x, Wq, bq, Wk, bk, Wv, bv)
    res = run_bass_kernel_spmd(nc, in_maps, core_ids=list(range(N_CORES)),
                               trace=_trace)
    out = assemble(res.results)
    if _trace:
        return out, res
    return out
